# revision 7
# baseline (speedup 1.0000x reference)
"""Trainium2 Bass kernel for an LSTM encoder-decoder chatbot model.

Model: question -> embed -> LSTM(512) -> linear(256) = q_out
       answer[:, :256] -> embed -> concat(q_out) -> LSTM(512) -> linear(32000)
Output: logits [B=32, W=32000, STEPS=256] f32.

Sharding: all 8 cores run the full (replicated) encoder + decoder
recurrence; the dominant 512x32000 output projection is sharded
column-wise (vocab) across cores; each core emits [32, 4000, 256].

Matmul strategy: hidden state kept transposed (hT fp16 [128, 4x32])
as the PE stationary operand; weights stream as the moving operand in
fp16. Four col-tiled matmuls (tile_position=(0,32c)) run concurrently,
one per 512-unit gate block, so the gates land on all 128 PSUM
partitions [(block,b), 512=i|f|g|o] and the elementwise LSTM cell runs
full-width. Gate columns are host-permuted accordingly.
"""
import sys
import numpy as np

sys.path.insert(0, '/opt/trn_rl_repo')

import concourse.bass as bass  # noqa: E402
import concourse.bacc as bacc  # noqa: E402
import concourse.mybir as mybir  # noqa: E402
import concourse.tile as tile  # noqa: E402
from concourse.bass import IndirectOffsetOnAxis  # noqa: E402
from concourse.bass_utils import run_bass_kernel_spmd  # noqa: E402

F32 = mybir.dt.float32
F16 = mybir.dt.float16
I32DT = mybir.dt.int32
AF = mybir.ActivationFunctionType

W_VOCAB = 32000
EMB = 256
STEPS = 256
HID = 512
QOUT = 256
B = 32
LQ = 50
NCORES = 8
VSH = W_VOCAB // NCORES      # 4000 vocab rows per core
VPAD = 4096                   # padded to 32 tiles of 128
G = 4 * HID                   # 2048 gate columns
TBLK = 64                     # decoder steps per hs block (4 blocks)

_cache = {}


def _gate_perm():
    """new gate col j = 512*blk + 128*gate + u  <-  old row 512*gate + 128*blk + u"""
    j = np.arange(G)
    blk, r = j // 512, j % 512
    gate, u = r // 128, r % 128
    return 512 * gate + 128 * blk + u


def build_program():
    nc = bacc.Bacc("TRN2", target_bir_lowering=False, debug=False,
                   num_devices=NCORES)

    def inp(name, shape, dt):
        return nc.dram_tensor(name, shape, dt, kind="ExternalInput").ap()

    q_idx = inp("q_idx", [13 * 128], I32DT)            # padded 1664
    a_idx = inp("a_idx", [STEPS * B], I32DT)           # 8192, t-major
    q_emb = inp("q_emb", [W_VOCAB, EMB], F16)
    a_emb = inp("a_emb", [W_VOCAB, EMB], F16)
    w_ihT_enc = inp("w_ihT_enc", [EMB, G], F16)        # permuted cols
    w_hhT_enc = inp("w_hhT_enc", [HID, G], F16)
    bias_enc = inp("bias_enc", [1, G], F16)
    w_ihAT = inp("w_ihAT", [EMB, G], F16)
    w_ihQT = inp("w_ihQT", [QOUT, G], F16)
    w_hhT_dec = inp("w_hhT_dec", [HID, G], F16)
    bias_dec = inp("bias_dec", [1, G], F16)
    q_lin_wT = inp("q_lin_wT", [HID, QOUT], F16)
    q_lin_b = inp("q_lin_b", [1, QOUT], F16)
    lin_wT = inp("lin_wT", [HID, VPAD], F16)           # per-core slice
    lin_b = inp("lin_b", [128, 32], F32)               # [u, mtile]
    i128f = inp("i128f", [128, 128], F32)
    i128h = inp("i128h", [128, 128], F16)
    i32h = inp("i32h", [32, 32], F16)
    ones1 = inp("ones1", [1, 32], F16)
    out = nc.dram_tensor("out", [B, VPAD, STEPS], F32,
                         kind="ExternalOutput").ap()

    with tile.TileContext(nc) as tc:
        _build(nc, tc, locals())
    nc.compile()
    return nc


def _build(nc, tc, t):
    from contextlib import ExitStack
    ctx = ExitStack()
    with ctx:
        _build_inner(nc, tc, t, ctx)


def _build_inner(nc, tc, t, ctx):
    # ---- pools -------------------------------------------------------
    wpool = ctx.enter_context(tc.tile_pool(name="weights", bufs=1))
    const = ctx.enter_context(tc.tile_pool(name="const", bufs=1))
    embp = ctx.enter_context(tc.tile_pool(name="embp", bufs=3))
    seqp = ctx.enter_context(tc.tile_pool(name="seqp", bufs=1))
    state = ctx.enter_context(tc.tile_pool(name="state", bufs=2))
    ew = ctx.enter_context(tc.tile_pool(name="ew", bufs=2))
    hsp = ctx.enter_context(tc.tile_pool(name="hsp", bufs=2))
    outp = ctx.enter_context(tc.tile_pool(name="outp", bufs=3))
    ps_g = ctx.enter_context(tc.tile_pool(name="ps_g", bufs=2, space="PSUM"))
    ps_tr = ctx.enter_context(tc.tile_pool(name="ps_tr", bufs=2, space="PSUM"))
    ps_p = ctx.enter_context(tc.tile_pool(name="ps_p", bufs=2, space="PSUM"))
    ps_m = ctx.enter_context(tc.tile_pool(name="ps_m", bufs=1, space="PSUM"))

    def load(pool, ap, dt=None, name=None):
        s = pool.tile(list(ap.shape), dt or ap.dtype, tag=name, name=name or 'ld')
        nc.sync.dma_start(s[:], ap[:])
        return s

    def loadc(pool, ap, name):
        p, cdim = ap.shape
        n = p // 128
        s = pool.tile([128, n * cdim], ap.dtype, tag=name, name=name)
        for k in range(n):
            nc.sync.dma_start(s[:, cdim * k:cdim * (k + 1)],
                              ap[128 * k:128 * (k + 1), :])
        def chunk(k, sl=slice(None)):
            base = cdim * k
            if sl == slice(None):
                return s[:, base:base + cdim]
            return s[:, base + sl.start:base + sl.stop]
        return chunk

    # ---- resident weights/constants ---------------------------------
    wih_e = loadc(wpool, t["w_ihT_enc"], "wih_e")     # 2 chunks [128,2048]
    whh_e = loadc(wpool, t["w_hhT_enc"], "whh_e")     # 4 chunks
    b_e = load(const, t["bias_enc"], name="b_e")
    wihA = loadc(wpool, t["w_ihAT"], "wihA")
    wihQ = loadc(wpool, t["w_ihQT"], "wihQ")
    whh_d = loadc(wpool, t["w_hhT_dec"], "whh_d")
    b_d = load(const, t["bias_dec"], name="b_d")
    qlw = loadc(wpool, t["q_lin_wT"], "qlw")          # 4 chunks [128,256]
    qlb = load(const, t["q_lin_b"], name="qlb")
    linw = loadc(wpool, t["lin_wT"], "linw")          # 4 chunks [128,4096]
    linb = load(const, t["lin_b"], name="linb")           # [128, 32] f32
    I128f = load(const, t["i128f"], name="I128f")
    I128h = load(const, t["i128h"], name="I128h")
    I32h = load(const, t["i32h"], name="I32h")
    ones = load(const, t["ones1"], name="ones")

    # index tiles
    qidx_sb = load(const, t["q_idx"].rearrange("(n p) -> n p", p=128)
                   .rearrange("n p -> p n"), name="qidx")   # [128, 13]
    aidx_sb = load(const, t["a_idx"].rearrange("(n p) -> n p", p=128)
                   .rearrange("n p -> p n"), name="aidx")   # [128, 64]

    # ---- embedding gather + transpose -> xT tiles --------------------
    def embed_T(table, idx_sb, ntiles, name):
        """gather rows (t-major) and transpose into xT [2 x [128, ntiles*128]] f16"""
        xT = [seqp.tile([128, ntiles * 128], F16, tag=f"{name}{k}", name=f"{name}{k}")
              for k in range(2)]
        for i in range(ntiles):
            rows = embp.tile([128, EMB], F16, tag="gather")
            nc.gpsimd.indirect_dma_start(
                out=rows[:], out_offset=None, in_=table[:],
                in_offset=IndirectOffsetOnAxis(ap=idx_sb[:, i:i + 1], axis=0))
            for k in range(2):
                p = ps_tr.tile([128, 128], F16, space="PSUM", tag="tr",
                               name="trp")
                nc.tensor.transpose(p[:], rows[:, 128 * k:128 * (k + 1)],
                                    I128h[:])
                nc.vector.tensor_copy(xT[k][:, 128 * i:128 * (i + 1)], p[:])
        return xT

    qT = embed_T(t["q_emb"], qidx_sb, 13, "qT")    # [256, 1664] f16
    aT = embed_T(t["a_emb"], aidx_sb, 64, "aT")    # [256, 8192] f16

    # ---- LSTM cell ---------------------------------------------------
    def step(hT, c_prev, seeds, wx_list, whh, has_h, want_hs):
        """One LSTM step, full-width col-tiled.

        seeds: list of (lhsT_ap[K,32], rhs_ap[K, 2048]) matmuls
        wx_list: list of (lhsT_ap, chunk_fn, k); whh: chunk accessor.
        """
        gp = ps_g.tile([128, 512], F32, space="PSUM", tag="gates")
        # rows = (lhsT_fn(sl), rhs_fn(sl)) emitted strip-innermost so the 4
        # col-strips run concurrently on the PE array
        rows = []
        for lhsT, rhs in seeds:
            rows.append((lambda sl, l=lhsT, r=rhs: (l, r[:, sl])))
        for lhsT, cf, k in wx_list:
            rows.append((lambda sl, l=lhsT, c2=cf, kk=k: (l, c2(kk, sl))))
        if has_h:
            for k in range(4):
                rows.append((lambda sl, kk=k: (hT[:, 32 * kk:32 * (kk + 1)],
                                               whh(kk, sl))))
        nrows = len(rows)
        for i, rowf in enumerate(rows):
            for c in range(4):
                sl = slice(512 * c, 512 * (c + 1))
                lhsT, rhs = rowf(sl)
                nc.tensor.matmul(gp[32 * c:32 * (c + 1), :], lhsT, rhs,
                                 start=(i == 0), stop=(i == nrows - 1),
                                 tile_position=(0, 32 * c))
        ig = ew.tile([128, 128], F32, tag="i")
        fg = ew.tile([128, 128], F32, tag="f")
        gg = ew.tile([128, 128], F32, tag="g")
        og = ew.tile([128, 128], F32, tag="o")
        nc.scalar.activation(ig[:], gp[:, 0:128], AF.Sigmoid)
        nc.scalar.activation(fg[:], gp[:, 128:256], AF.Sigmoid)
        nc.scalar.activation(gg[:], gp[:, 256:384], AF.Tanh)
        nc.scalar.activation(og[:], gp[:, 384:512], AF.Sigmoid)
        igg = ew.tile([128, 128], F32, tag="ig")
        nc.vector.tensor_mul(igg[:], ig[:], gg[:])
        c_new = state.tile([128, 128], F32, tag="c")
        if c_prev is None:
            nc.vector.tensor_copy(c_new[:], igg[:])  # c0 = 0 -> c = i*g
        else:
            fc = ew.tile([128, 128], F32, tag="fc")
            nc.vector.tensor_mul(fc[:], fg[:], c_prev[:])
            nc.vector.tensor_add(c_new[:], igg[:], fc[:])
        th = ew.tile([128, 128], F32, tag="th")
        nc.scalar.activation(th[:], c_new[:], AF.Tanh)
        h_new = ew.tile([128, 128], F32, tag="h")
        nc.vector.tensor_mul(h_new[:], og[:], th[:])
        trp = ps_tr.tile([128, 128], F32, space="PSUM", tag="tr", name="trh")
        nc.tensor.transpose(trp[:], h_new[:], I128f[:])
        hT_new = state.tile([128, 128], F16, tag="hT")
        nc.vector.tensor_copy(hT_new[:], trp[:])
        return hT_new, c_new

    # ---- encoder -----------------------------------------------------
    hT = None
    c = None
    for tt in range(LQ):
        sl32 = slice(32 * tt, 32 * (tt + 1))
        seeds = [(ones[:], b_e[:])]
        wx = [(qT[0][:, sl32], wih_e, 0),
              (qT[1][:, sl32], wih_e, 1)]
        hT, c = step(hT, c, seeds, wx, whh_e, has_h=(tt > 0), want_hs=False)

    # ---- q_out = h @ q_lin_w.T + b; then Qb = q_out @ w_ihQ.T + bias_dec
    qo_p_t = ps_m.tile([32, 512], F32, space="PSUM", tag="misc", name="qo_p")
    qo_p = qo_p_t[:, 0:QOUT]
    nc.tensor.matmul(qo_p[:], ones[:], qlb[:], start=True, stop=False)
    for k in range(4):
        nc.tensor.matmul(qo_p[:], hT[:, 32 * k:32 * (k + 1)],
                         qlw(k), start=False, stop=(k == 3))
    qo = seqp.tile([32, QOUT], F16, tag="qo_sb")
    nc.scalar.activation(qo[:], qo_p[:], AF.Identity)
    # transpose q_out [32,256] -> [256(2x128), 32] f16
    qoT = seqp.tile([128, 64], F16, tag="qoT")
    for k in range(2):
        p = ps_tr.tile([128, 128], F16, space="PSUM", tag="tr", name="trq")
        nc.tensor.transpose(p[:, 0:32], qo[:, 128 * k:128 * (k + 1)], I32h[:])
        nc.vector.tensor_copy(qoT[:, 32 * k:32 * (k + 1)], p[:, 0:32])
    # Qb [32, 2048] f16, quarter at a time (no col tiling, partition 0-31)
    qb = seqp.tile([32, G], F16, tag="qb")
    for qtr in range(4):
        sl = slice(512 * qtr, 512 * (qtr + 1))
        qp = ps_m.tile([32, 512], F32, space="PSUM", tag="misc")
        nc.tensor.matmul(qp[:], ones[:], b_d[:, sl], start=True, stop=False)
        for k in range(2):
            nc.tensor.matmul(qp[:], qoT[:, 32 * k:32 * (k + 1)],
                             wihQ(k, sl), start=False, stop=(k == 1))
        nc.scalar.activation(qb[:, sl], qp[:], AF.Identity)

    # ---- decoder + projection, interleaved per TBLK ------------------
    out = t["out"]
    for blk in range(STEPS // TBLK):
        hs = [hsp.tile([128, TBLK * 32], F16, tag=f"hs{k}", name=f"hs{k}") for k in range(4)]
        for dt in range(TBLK):
            tt = blk * TBLK + dt
            sl32 = slice(32 * tt, 32 * (tt + 1))
            seeds = [(I32h[:], qb[:])]
            wx = [(aT[0][:, sl32], wihA, 0),
                  (aT[1][:, sl32], wihA, 1)]
            hT, c = step(hT, c, seeds, wx, whh_d, has_h=True, want_hs=True)
            # scatter hT into hs block tiles: chunk k cols {b*TBLK+dt}
            for k in range(4):
                dst = hs[k].rearrange("p (b t) -> p b t", b=32)[:, :, dt]
                nc.gpsimd.tensor_copy(dst, hT[:, 32 * k:32 * (k + 1)])
        # projection of this block: tokens = 32*TBLK, b-major cols
        for m in range(VPAD // 128):
            for s in range(TBLK * 32 // 512):
                pp = ps_p.tile([128, 512], F32, space="PSUM", tag="proj")
                for k in range(4):
                    nc.tensor.matmul(
                        pp[:], linw(k, slice(128 * m, 128 * (m + 1))),
                        hs[k][:, 512 * s:512 * (s + 1)],
                        start=(k == 0), stop=(k == 3))
                ot = outp.tile([128, 512], F32, tag="ot")
                nc.scalar.activation(ot[:], pp[:], AF.Identity,
                                     bias=linb[:, m:m + 1])
                nb = 512 // TBLK  # batches per sub-block
                dst = out[nb * s:nb * (s + 1), 128 * m:128 * (m + 1),
                          blk * TBLK:(blk + 1) * TBLK].rearrange(
                              "b w t -> w b t")
                nc.sync.dma_start(dst, ot[:].rearrange(
                    "w (b t) -> w b t", b=nb))


def kernel(**inputs):
    inp = {k: np.asarray(v) for k, v in inputs.items()}
    if "prog" not in _cache:
        _cache["prog"] = build_program()
    nc = _cache["prog"]

    perm = _gate_perm()
    f16 = np.float16

    def prep_lstm(w_ih, w_hh, b_ih, b_hh):
        wihT = np.ascontiguousarray(w_ih.T[:, perm]).astype(f16)
        whhT = np.ascontiguousarray(w_hh.T[:, perm]).astype(f16)
        bias = (b_ih + b_hh)[perm][None, :].astype(f16)
        return wihT, whhT, bias

    wihT_e, whhT_e, b_e = prep_lstm(inp["q_lstm_w_ih"], inp["q_lstm_w_hh"],
                                    inp["q_lstm_b_ih"], inp["q_lstm_b_hh"])
    wihT_d, whhT_d, b_d = prep_lstm(inp["a_lstm_w_ih"], inp["a_lstm_w_hh"],
                                    inp["a_lstm_b_ih"], inp["a_lstm_b_hh"])
    wihAT = np.ascontiguousarray(wihT_d[:EMB])
    wihQT = np.ascontiguousarray(wihT_d[EMB:])

    q_idx = np.zeros(13 * 128, np.int32)
    q_idx[:B * LQ] = inp["question"].T.reshape(-1).astype(np.int32)
    a_idx = inp["answer"][:, :STEPS].T.reshape(-1).astype(np.int32)

    lin_w = inp["lin_w"].astype(np.float32)   # [32000, 512]
    lin_b = inp["lin_b"].astype(np.float32)

    base = {
        "q_idx": q_idx, "a_idx": a_idx,
        "q_emb": inp["q_emb_w"].astype(f16),
        "a_emb": inp["a_emb_w"].astype(f16),
        "w_ihT_enc": wihT_e, "w_hhT_enc": whhT_e, "bias_enc": b_e,
        "w_ihAT": wihAT, "w_ihQT": wihQT, "w_hhT_dec": whhT_d,
        "bias_dec": b_d,
        "q_lin_wT": np.ascontiguousarray(inp["q_lin_w"].T).astype(f16),
        "q_lin_b": inp["q_lin_b"][None, :].astype(f16),
        "i128f": np.eye(128, dtype=np.float32),
        "i128h": np.eye(128, dtype=f16),
        "i32h": np.eye(32, dtype=f16),
        "ones1": np.ones((1, 32), f16),
    }
    in_maps = []
    for core in range(NCORES):
        m = dict(base)
        sl = lin_w[VSH * core: VSH * (core + 1)]          # [4000, 512]
        slp = np.zeros((VPAD, HID), np.float32)
        slp[:VSH] = sl
        m["lin_wT"] = np.ascontiguousarray(slp.T).astype(f16)
        bp = np.zeros(VPAD, np.float32)
        bp[:VSH] = lin_b[VSH * core: VSH * (core + 1)]
        m["lin_b"] = np.ascontiguousarray(bp.reshape(32, 128).T)
        in_maps.append(m)

    _cache["in_maps"] = in_maps
    res = run_bass_kernel_spmd(nc, in_maps, core_ids=list(range(NCORES)))
    _cache["last_res"] = res
    out = np.concatenate(
        [res.results[i]["out"][:, :VSH, :] for i in range(NCORES)], axis=1)
    return out.astype(np.float32)


if __name__ == "__main__":
    import reference
    ins = reference.setup_inputs()
    ref = np.asarray(reference.reference(**ins))
    got = kernel(**{k: np.asarray(v) for k, v in ins.items()})
    err = np.abs(got - ref).max() / (np.abs(ref).max() + 1e-12)
    print("max abs err:", np.abs(got - ref).max(), "rel:", err)


def run_traced():
    nc = _cache["prog"]
    return run_bass_kernel_spmd(nc, _cache["in_maps"],
                                core_ids=list(range(NCORES)), trace=True)


# revision 8
# speedup vs baseline: 1.0000x; 1.0000x over previous
"""Trainium2 Bass kernel for an LSTM encoder-decoder chatbot model.

Model: question -> embed -> LSTM(512) -> linear(256) = q_out
       answer[:, :256] -> embed -> concat(q_out) -> LSTM(512) -> linear(32000)
Output: logits [B=32, W=32000, STEPS=256] f32.

Sharding: all 8 cores run the full (replicated) encoder + decoder
recurrence; the dominant 512x32000 output projection is sharded
column-wise (vocab) across cores; each core emits [32, 4000, 256].

Matmul strategy: hidden state kept transposed (hT fp16 [128, 4x32])
as the PE stationary operand; weights stream as the moving operand in
fp16. Four col-tiled matmuls (tile_position=(0,32c)) run concurrently,
one per 512-unit gate block, so the gates land on all 128 PSUM
partitions [(block,b), 512=i|f|g|o] and the elementwise LSTM cell runs
full-width. Gate columns are host-permuted accordingly.
"""
import sys
import numpy as np

sys.path.insert(0, '/opt/trn_rl_repo')

import concourse.bass as bass  # noqa: E402
import concourse.bacc as bacc  # noqa: E402
import concourse.mybir as mybir  # noqa: E402
import concourse.tile as tile  # noqa: E402
from concourse.bass import IndirectOffsetOnAxis  # noqa: E402
from concourse.bass_utils import run_bass_kernel_spmd  # noqa: E402

F32 = mybir.dt.float32
F16 = mybir.dt.float16
I32DT = mybir.dt.int32
AF = mybir.ActivationFunctionType

W_VOCAB = 32000
EMB = 256
STEPS = 256
HID = 512
QOUT = 256
B = 32
LQ = 50
NCORES = 8
VSH = W_VOCAB // NCORES      # 4000 vocab rows per core
VPAD = 4096                   # padded to 32 tiles of 128
G = 4 * HID                   # 2048 gate columns
TBLK = 64                     # decoder steps per hs block (4 blocks)

_cache = {}


def _gate_perm():
    """new gate col j = 512*blk + 128*gate + u  <-  old row 512*gate + 128*blk + u"""
    j = np.arange(G)
    blk, r = j // 512, j % 512
    gate, u = r // 128, r % 128
    return 512 * gate + 128 * blk + u


def build_program():
    nc = bacc.Bacc("TRN2", target_bir_lowering=False, debug=False,
                   num_devices=NCORES)

    def inp(name, shape, dt):
        return nc.dram_tensor(name, shape, dt, kind="ExternalInput").ap()

    q_idx = inp("q_idx", [13 * 128], I32DT)            # padded 1664
    a_idx = inp("a_idx", [STEPS * B], I32DT)           # 8192, t-major
    q_emb = inp("q_emb", [W_VOCAB, EMB], F16)
    a_emb = inp("a_emb", [W_VOCAB, EMB], F16)
    w_ihT_enc = inp("w_ihT_enc", [EMB, G], F16)        # permuted cols
    w_hhT_enc = inp("w_hhT_enc", [HID, G], F16)
    bias_enc = inp("bias_enc", [1, G], F16)
    w_ihAT = inp("w_ihAT", [EMB, G], F16)
    w_ihQT = inp("w_ihQT", [QOUT, G], F16)
    w_hhT_dec = inp("w_hhT_dec", [HID, G], F16)
    bias_dec = inp("bias_dec", [1, G], F16)
    q_lin_wT = inp("q_lin_wT", [HID, QOUT], F16)
    q_lin_b = inp("q_lin_b", [1, QOUT], F16)
    lin_wT = inp("lin_wT", [HID, VPAD], F16)           # per-core slice
    lin_b = inp("lin_b", [128, 32], F32)               # [u, mtile]
    i128f = inp("i128f", [128, 128], F32)
    i128h = inp("i128h", [128, 128], F16)
    i32h = inp("i32h", [32, 32], F16)
    ones1 = inp("ones1", [1, 32], F16)
    out = nc.dram_tensor("out", [B, VPAD, STEPS], F32,
                         kind="ExternalOutput").ap()

    with tile.TileContext(nc) as tc:
        _build(nc, tc, locals())
    nc.compile()
    return nc


def _build(nc, tc, t):
    from contextlib import ExitStack
    ctx = ExitStack()
    with ctx:
        _build_inner(nc, tc, t, ctx)


def _build_inner(nc, tc, t, ctx):
    # ---- pools -------------------------------------------------------
    wpool = ctx.enter_context(tc.tile_pool(name="weights", bufs=1))
    const = ctx.enter_context(tc.tile_pool(name="const", bufs=1))
    embp = ctx.enter_context(tc.tile_pool(name="embp", bufs=3))
    seqp = ctx.enter_context(tc.tile_pool(name="seqp", bufs=1))
    state = ctx.enter_context(tc.tile_pool(name="state", bufs=3))
    ew = ctx.enter_context(tc.tile_pool(name="ew", bufs=3))
    hsp = ctx.enter_context(tc.tile_pool(name="hsp", bufs=2))
    outp = ctx.enter_context(tc.tile_pool(name="outp", bufs=3))
    ps_g = ctx.enter_context(tc.tile_pool(name="ps_g", bufs=2, space="PSUM"))
    ps_tr = ctx.enter_context(tc.tile_pool(name="ps_tr", bufs=2, space="PSUM"))
    ps_p = ctx.enter_context(tc.tile_pool(name="ps_p", bufs=2, space="PSUM"))
    ps_m = ctx.enter_context(tc.tile_pool(name="ps_m", bufs=1, space="PSUM"))

    def load(pool, ap, dt=None, name=None):
        s = pool.tile(list(ap.shape), dt or ap.dtype, tag=name, name=name or 'ld')
        nc.sync.dma_start(s[:], ap[:])
        return s

    def loadc(pool, ap, name):
        p, cdim = ap.shape
        n = p // 128
        s = pool.tile([128, n * cdim], ap.dtype, tag=name, name=name)
        for k in range(n):
            nc.sync.dma_start(s[:, cdim * k:cdim * (k + 1)],
                              ap[128 * k:128 * (k + 1), :])
        def chunk(k, sl=slice(None)):
            base = cdim * k
            if sl == slice(None):
                return s[:, base:base + cdim]
            return s[:, base + sl.start:base + sl.stop]
        return chunk

    # ---- resident weights/constants ---------------------------------
    wih_e = loadc(wpool, t["w_ihT_enc"], "wih_e")     # 2 chunks [128,2048]
    whh_e = loadc(wpool, t["w_hhT_enc"], "whh_e")     # 4 chunks
    b_e = load(const, t["bias_enc"], name="b_e")
    wihA = loadc(wpool, t["w_ihAT"], "wihA")
    wihQ = loadc(wpool, t["w_ihQT"], "wihQ")
    whh_d = loadc(wpool, t["w_hhT_dec"], "whh_d")
    b_d = load(const, t["bias_dec"], name="b_d")
    qlw = loadc(wpool, t["q_lin_wT"], "qlw")          # 4 chunks [128,256]
    qlb = load(const, t["q_lin_b"], name="qlb")
    linw = loadc(wpool, t["lin_wT"], "linw")          # 4 chunks [128,4096]
    linb = load(const, t["lin_b"], name="linb")           # [128, 32] f32
    I128f = load(const, t["i128f"], name="I128f")
    I128h = load(const, t["i128h"], name="I128h")
    I32h = load(const, t["i32h"], name="I32h")
    ones = load(const, t["ones1"], name="ones")

    # index tiles
    qidx_sb = load(const, t["q_idx"].rearrange("(n p) -> n p", p=128)
                   .rearrange("n p -> p n"), name="qidx")   # [128, 13]
    aidx_sb = load(const, t["a_idx"].rearrange("(n p) -> n p", p=128)
                   .rearrange("n p -> p n"), name="aidx")   # [128, 64]

    # ---- embedding gather + transpose -> xT tiles --------------------
    def embed_T(table, idx_sb, ntiles, name):
        """gather rows (t-major) and transpose into xT [2 x [128, ntiles*128]] f16"""
        xT = [seqp.tile([128, ntiles * 128], F16, tag=f"{name}{k}", name=f"{name}{k}")
              for k in range(2)]
        for i in range(ntiles):
            rows = embp.tile([128, EMB], F16, tag="gather")
            nc.gpsimd.indirect_dma_start(
                out=rows[:], out_offset=None, in_=table[:],
                in_offset=IndirectOffsetOnAxis(ap=idx_sb[:, i:i + 1], axis=0))
            for k in range(2):
                p = ps_tr.tile([128, 128], F16, space="PSUM", tag="tr",
                               name="trp")
                nc.tensor.transpose(p[:], rows[:, 128 * k:128 * (k + 1)],
                                    I128h[:])
                nc.vector.tensor_copy(xT[k][:, 128 * i:128 * (i + 1)], p[:])
        return xT

    qT = embed_T(t["q_emb"], qidx_sb, 13, "qT")    # [256, 1664] f16
    aT = embed_T(t["a_emb"], aidx_sb, 64, "aT")    # [256, 8192] f16

    # ---- LSTM cell ---------------------------------------------------
    def step(hT, c_prev, seeds, wx_list, whh, has_h, want_hs):
        """One LSTM step, full-width col-tiled.

        seeds: list of (lhsT_ap[K,32], rhs_ap[K, 2048]) matmuls
        wx_list: list of (lhsT_ap, chunk_fn, k); whh: chunk accessor.
        """
        gp = ps_g.tile([128, 512], F32, space="PSUM", tag="gates")
        # rows = (lhsT_fn(sl), rhs_fn(sl)) emitted strip-innermost so the 4
        # col-strips run concurrently on the PE array
        rows = []
        for lhsT, rhs in seeds:
            rows.append((lambda sl, l=lhsT, r=rhs: (l, r[:, sl])))
        for lhsT, cf, k in wx_list:
            rows.append((lambda sl, l=lhsT, c2=cf, kk=k: (l, c2(kk, sl))))
        if has_h:
            for k in range(4):
                rows.append((lambda sl, kk=k: (hT[:, 32 * kk:32 * (kk + 1)],
                                               whh(kk, sl))))
        nrows = len(rows)
        for i, rowf in enumerate(rows):
            for c in range(4):
                sl = slice(512 * c, 512 * (c + 1))
                lhsT, rhs = rowf(sl)
                nc.tensor.matmul(gp[32 * c:32 * (c + 1), :], lhsT, rhs,
                                 start=(i == 0), stop=(i == nrows - 1),
                                 tile_position=(0, 32 * c))
        ig = ew.tile([128, 128], F32, tag="i")
        fg = ew.tile([128, 128], F32, tag="f")
        gg = ew.tile([128, 128], F32, tag="g")
        og = ew.tile([128, 128], F32, tag="o")
        nc.scalar.activation(ig[:], gp[:, 0:128], AF.Sigmoid)
        nc.scalar.activation(fg[:], gp[:, 128:256], AF.Sigmoid)
        nc.scalar.activation(gg[:], gp[:, 256:384], AF.Tanh)
        nc.scalar.activation(og[:], gp[:, 384:512], AF.Sigmoid)
        igg = ew.tile([128, 128], F32, tag="ig")
        nc.vector.tensor_mul(igg[:], ig[:], gg[:])
        c_new = state.tile([128, 128], F32, tag="c")
        if c_prev is None:
            nc.vector.tensor_copy(c_new[:], igg[:])  # c0 = 0 -> c = i*g
        else:
            fc = ew.tile([128, 128], F32, tag="fc")
            nc.vector.tensor_mul(fc[:], fg[:], c_prev[:])
            nc.vector.tensor_add(c_new[:], igg[:], fc[:])
        th = ew.tile([128, 128], F32, tag="th")
        nc.scalar.activation(th[:], c_new[:], AF.Tanh)
        h_new = ew.tile([128, 128], F32, tag="h")
        nc.vector.tensor_mul(h_new[:], og[:], th[:])
        trp = ps_tr.tile([128, 128], F32, space="PSUM", tag="tr", name="trh")
        nc.tensor.transpose(trp[:], h_new[:], I128f[:])
        hT_new = state.tile([128, 128], F16, tag="hT")
        nc.vector.tensor_copy(hT_new[:], trp[:])
        return hT_new, c_new

    # ---- encoder -----------------------------------------------------
    hT = None
    c = None
    for tt in range(LQ):
        sl32 = slice(32 * tt, 32 * (tt + 1))
        seeds = [(ones[:], b_e[:])]
        wx = [(qT[0][:, sl32], wih_e, 0),
              (qT[1][:, sl32], wih_e, 1)]
        hT, c = step(hT, c, seeds, wx, whh_e, has_h=(tt > 0), want_hs=False)

    # ---- q_out = h @ q_lin_w.T + b; then Qb = q_out @ w_ihQ.T + bias_dec
    qo_p_t = ps_m.tile([32, 512], F32, space="PSUM", tag="misc", name="qo_p")
    qo_p = qo_p_t[:, 0:QOUT]
    nc.tensor.matmul(qo_p[:], ones[:], qlb[:], start=True, stop=False)
    for k in range(4):
        nc.tensor.matmul(qo_p[:], hT[:, 32 * k:32 * (k + 1)],
                         qlw(k), start=False, stop=(k == 3))
    qo = seqp.tile([32, QOUT], F16, tag="qo_sb")
    nc.scalar.activation(qo[:], qo_p[:], AF.Identity)
    # transpose q_out [32,256] -> [256(2x128), 32] f16
    qoT = seqp.tile([128, 64], F16, tag="qoT")
    for k in range(2):
        p = ps_tr.tile([128, 128], F16, space="PSUM", tag="tr", name="trq")
        nc.tensor.transpose(p[:, 0:32], qo[:, 128 * k:128 * (k + 1)], I32h[:])
        nc.vector.tensor_copy(qoT[:, 32 * k:32 * (k + 1)], p[:, 0:32])
    # Qb [32, 2048] f16, quarter at a time (no col tiling, partition 0-31)
    qb = seqp.tile([32, G], F16, tag="qb")
    for qtr in range(4):
        sl = slice(512 * qtr, 512 * (qtr + 1))
        qp = ps_m.tile([32, 512], F32, space="PSUM", tag="misc")
        nc.tensor.matmul(qp[:], ones[:], b_d[:, sl], start=True, stop=False)
        for k in range(2):
            nc.tensor.matmul(qp[:], qoT[:, 32 * k:32 * (k + 1)],
                             wihQ(k, sl), start=False, stop=(k == 1))
        nc.scalar.activation(qb[:, sl], qp[:], AF.Identity)

    # ---- decoder + projection, interleaved per TBLK ------------------
    out = t["out"]
    for blk in range(STEPS // TBLK):
        hs = [hsp.tile([128, TBLK * 32], F16, tag=f"hs{k}", name=f"hs{k}") for k in range(4)]
        for dt in range(TBLK):
            tt = blk * TBLK + dt
            sl32 = slice(32 * tt, 32 * (tt + 1))
            seeds = [(I32h[:], qb[:])]
            wx = [(aT[0][:, sl32], wihA, 0),
                  (aT[1][:, sl32], wihA, 1)]
            hT, c = step(hT, c, seeds, wx, whh_d, has_h=True, want_hs=True)
            # scatter hT into hs block tiles: chunk k cols {b*TBLK+dt}
            for k in range(4):
                dst = hs[k].rearrange("p (b t) -> p b t", b=32)[:, :, dt]
                nc.gpsimd.tensor_copy(dst, hT[:, 32 * k:32 * (k + 1)])
        # projection of this block: tokens = 32*TBLK, b-major cols
        for m in range(VPAD // 128):
            for s in range(TBLK * 32 // 512):
                pp = ps_p.tile([128, 512], F32, space="PSUM", tag="proj")
                for k in range(4):
                    nc.tensor.matmul(
                        pp[:], linw(k, slice(128 * m, 128 * (m + 1))),
                        hs[k][:, 512 * s:512 * (s + 1)],
                        start=(k == 0), stop=(k == 3))
                ot = outp.tile([128, 512], F32, tag="ot")
                nc.scalar.activation(ot[:], pp[:], AF.Identity,
                                     bias=linb[:, m:m + 1])
                nb = 512 // TBLK  # batches per sub-block
                dst = out[nb * s:nb * (s + 1), 128 * m:128 * (m + 1),
                          blk * TBLK:(blk + 1) * TBLK].rearrange(
                              "b w t -> w b t")
                nc.sync.dma_start(dst, ot[:].rearrange(
                    "w (b t) -> w b t", b=nb))


def kernel(**inputs):
    inp = {k: np.asarray(v) for k, v in inputs.items()}
    if "prog" not in _cache:
        _cache["prog"] = build_program()
    nc = _cache["prog"]

    perm = _gate_perm()
    f16 = np.float16

    def prep_lstm(w_ih, w_hh, b_ih, b_hh):
        wihT = np.ascontiguousarray(w_ih.T[:, perm]).astype(f16)
        whhT = np.ascontiguousarray(w_hh.T[:, perm]).astype(f16)
        bias = (b_ih + b_hh)[perm][None, :].astype(f16)
        return wihT, whhT, bias

    wihT_e, whhT_e, b_e = prep_lstm(inp["q_lstm_w_ih"], inp["q_lstm_w_hh"],
                                    inp["q_lstm_b_ih"], inp["q_lstm_b_hh"])
    wihT_d, whhT_d, b_d = prep_lstm(inp["a_lstm_w_ih"], inp["a_lstm_w_hh"],
                                    inp["a_lstm_b_ih"], inp["a_lstm_b_hh"])
    wihAT = np.ascontiguousarray(wihT_d[:EMB])
    wihQT = np.ascontiguousarray(wihT_d[EMB:])

    q_idx = np.zeros(13 * 128, np.int32)
    q_idx[:B * LQ] = inp["question"].T.reshape(-1).astype(np.int32)
    a_idx = inp["answer"][:, :STEPS].T.reshape(-1).astype(np.int32)

    lin_w = inp["lin_w"].astype(np.float32)   # [32000, 512]
    lin_b = inp["lin_b"].astype(np.float32)

    base = {
        "q_idx": q_idx, "a_idx": a_idx,
        "q_emb": inp["q_emb_w"].astype(f16),
        "a_emb": inp["a_emb_w"].astype(f16),
        "w_ihT_enc": wihT_e, "w_hhT_enc": whhT_e, "bias_enc": b_e,
        "w_ihAT": wihAT, "w_ihQT": wihQT, "w_hhT_dec": whhT_d,
        "bias_dec": b_d,
        "q_lin_wT": np.ascontiguousarray(inp["q_lin_w"].T).astype(f16),
        "q_lin_b": inp["q_lin_b"][None, :].astype(f16),
        "i128f": np.eye(128, dtype=np.float32),
        "i128h": np.eye(128, dtype=f16),
        "i32h": np.eye(32, dtype=f16),
        "ones1": np.ones((1, 32), f16),
    }
    in_maps = []
    for core in range(NCORES):
        m = dict(base)
        sl = lin_w[VSH * core: VSH * (core + 1)]          # [4000, 512]
        slp = np.zeros((VPAD, HID), np.float32)
        slp[:VSH] = sl
        m["lin_wT"] = np.ascontiguousarray(slp.T).astype(f16)
        bp = np.zeros(VPAD, np.float32)
        bp[:VSH] = lin_b[VSH * core: VSH * (core + 1)]
        m["lin_b"] = np.ascontiguousarray(bp.reshape(32, 128).T)
        in_maps.append(m)

    _cache["in_maps"] = in_maps
    res = run_bass_kernel_spmd(nc, in_maps, core_ids=list(range(NCORES)))
    _cache["last_res"] = res
    out = np.concatenate(
        [res.results[i]["out"][:, :VSH, :] for i in range(NCORES)], axis=1)
    return out.astype(np.float32)


if __name__ == "__main__":
    import reference
    ins = reference.setup_inputs()
    ref = np.asarray(reference.reference(**ins))
    got = kernel(**{k: np.asarray(v) for k, v in ins.items()})
    err = np.abs(got - ref).max() / (np.abs(ref).max() + 1e-12)
    print("max abs err:", np.abs(got - ref).max(), "rel:", err)


def run_traced():
    nc = _cache["prog"]
    return run_bass_kernel_spmd(nc, _cache["in_maps"],
                                core_ids=list(range(NCORES)), trace=True)


# revision 15
# speedup vs baseline: 1.0741x; 1.0741x over previous
"""Trainium2 Bass kernel for an LSTM encoder-decoder chatbot model.

Model: question -> embed -> LSTM(512) -> linear(256) = q_out
       answer[:, :256] -> embed -> concat(q_out) -> LSTM(512) -> linear(32000)
Output: logits [B=32, W=32000, STEPS=256] f32.

Sharding: all 8 cores run the full (replicated) encoder + decoder
recurrence; the dominant 512x32000 output projection is sharded
column-wise (vocab) across cores; each core emits [32, 4000, 256].

Matmul strategy: hidden state kept transposed (hT fp16 [128, 4x32])
as the PE stationary operand; weights stream as the moving operand in
fp16. Four col-tiled matmuls (tile_position=(0,32c)) run concurrently,
one per 512-unit gate block, so the gates land on all 128 PSUM
partitions [(block,b), 512=i|f|g|o] and the elementwise LSTM cell runs
full-width. Gate columns are host-permuted accordingly.
"""
import sys
import numpy as np

sys.path.insert(0, '/opt/trn_rl_repo')

import concourse.bass as bass  # noqa: E402
import concourse.bacc as bacc  # noqa: E402
import concourse.mybir as mybir  # noqa: E402
import concourse.tile as tile  # noqa: E402
from concourse.bass import IndirectOffsetOnAxis  # noqa: E402
from concourse.bass_utils import run_bass_kernel_spmd  # noqa: E402

F32 = mybir.dt.float32
F16 = mybir.dt.float16
I32DT = mybir.dt.int32
AF = mybir.ActivationFunctionType

W_VOCAB = 32000
EMB = 256
STEPS = 256
HID = 512
QOUT = 256
B = 32
LQ = 50
NCORES = 8
VSH = W_VOCAB // NCORES      # 4000 vocab rows per core
VPAD = 4096                   # padded to 32 tiles of 128
G = 4 * HID                   # 2048 gate columns
TBLK = 32                     # decoder steps per hs block (8 blocks)

_cache = {}


def _gate_perm():
    """Block layout [i|f|o|g]x128 per 128-unit block: new col
    j = 512*blk + 128*slot + u  <-  old row 512*gate + 128*blk + u,
    with slot order (i, f, o, g) so the three sigmoids are contiguous."""
    j = np.arange(G)
    blk, r = j // 512, j % 512
    slot, u = r // 128, r % 128
    old_gate = np.array([0, 1, 3, 2])[slot]
    return 512 * old_gate + 128 * blk + u


def build_program():
    nc = bacc.Bacc("TRN2", target_bir_lowering=False, debug=False,
                   num_devices=NCORES)

    def inp(name, shape, dt):
        return nc.dram_tensor(name, shape, dt, kind="ExternalInput").ap()

    q_idx = inp("q_idx", [13 * 128], I32DT)            # padded 1664
    a_idx = inp("a_idx", [STEPS * B], I32DT)           # 8192, t-major
    q_emb = inp("q_emb", [W_VOCAB, EMB], F16)
    a_emb = inp("a_emb", [W_VOCAB, EMB], F16)
    w_ihT_enc = inp("w_ihT_enc", [EMB, G], F16)        # permuted cols
    w_hhT_enc = inp("w_hhT_enc", [HID, G], F16)
    bias_enc = inp("bias_enc", [1, G], F16)
    w_ihAT = inp("w_ihAT", [EMB, G], F16)
    w_ihQT = inp("w_ihQT", [QOUT, G], F16)
    w_hhT_dec = inp("w_hhT_dec", [HID, G], F16)
    bias_dec = inp("bias_dec", [1, G], F16)
    q_lin_wT = inp("q_lin_wT", [HID, QOUT], F16)
    q_lin_b = inp("q_lin_b", [1, QOUT], F16)
    lin_wT = inp("lin_wT", [HID, VPAD], F16)           # per-core slice
    lin_b = inp("lin_b", [128, 32], F32)               # [u, mtile]
    i128f = inp("i128f", [128, 128], F32)
    i128h = inp("i128h", [128, 128], F16)
    i32h = inp("i32h", [32, 32], F16)
    ones1 = inp("ones1", [1, 32], F16)
    out = nc.dram_tensor("out", [B, VPAD, STEPS], F32,
                         kind="ExternalOutput").ap()

    with tile.TileContext(nc) as tc:
        _build(nc, tc, locals())
    nc.compile()
    return nc


def _build(nc, tc, t):
    from contextlib import ExitStack
    ctx = ExitStack()
    with ctx:
        _build_inner(nc, tc, t, ctx)


def _build_inner(nc, tc, t, ctx):
    # ---- pools -------------------------------------------------------
    wpool = ctx.enter_context(tc.tile_pool(name="weights", bufs=1))
    const = ctx.enter_context(tc.tile_pool(name="const", bufs=1))
    embp = ctx.enter_context(tc.tile_pool(name="embp", bufs=3))
    seqp = ctx.enter_context(tc.tile_pool(name="seqp", bufs=1))
    state = ctx.enter_context(tc.tile_pool(name="state", bufs=3))
    ew = ctx.enter_context(tc.tile_pool(name="ew", bufs=3))
    hsp = ctx.enter_context(tc.tile_pool(name="hsp", bufs=2))
    outp = ctx.enter_context(tc.tile_pool(name="outp", bufs=3))
    ps_g = ctx.enter_context(tc.tile_pool(name="ps_g", bufs=3, space="PSUM"))
    ps_tr = ctx.enter_context(tc.tile_pool(name="ps_tr", bufs=2, space="PSUM"))
    ps_p = ctx.enter_context(tc.tile_pool(name="ps_p", bufs=2, space="PSUM"))
    ps_m = ctx.enter_context(tc.tile_pool(name="ps_m", bufs=1, space="PSUM"))

    def load(pool, ap, dt=None, name=None):
        s = pool.tile(list(ap.shape), dt or ap.dtype, tag=name, name=name or 'ld')
        nc.sync.dma_start(s[:], ap[:])
        return s

    def loadc(pool, ap, name):
        p, cdim = ap.shape
        n = p // 128
        s = pool.tile([128, n * cdim], ap.dtype, tag=name, name=name)
        for k in range(n):
            nc.sync.dma_start(s[:, cdim * k:cdim * (k + 1)],
                              ap[128 * k:128 * (k + 1), :])
        def chunk(k, sl=slice(None)):
            base = cdim * k
            if sl == slice(None):
                return s[:, base:base + cdim]
            return s[:, base + sl.start:base + sl.stop]
        return chunk

    # ---- resident weights/constants ---------------------------------
    wih_e = loadc(wpool, t["w_ihT_enc"], "wih_e")     # 2 chunks [128,2048]
    whh_e = loadc(wpool, t["w_hhT_enc"], "whh_e")     # 4 chunks
    b_e = load(const, t["bias_enc"], name="b_e")
    wihA = loadc(wpool, t["w_ihAT"], "wihA")
    wihQ = loadc(wpool, t["w_ihQT"], "wihQ")
    whh_d = loadc(wpool, t["w_hhT_dec"], "whh_d")
    b_d = load(const, t["bias_dec"], name="b_d")
    qlw = loadc(wpool, t["q_lin_wT"], "qlw")          # 4 chunks [128,256]
    qlb = load(const, t["q_lin_b"], name="qlb")
    linw = loadc(wpool, t["lin_wT"], "linw")          # 4 chunks [128,4096]
    linb = load(const, t["lin_b"], name="linb")           # [128, 32] f32
    I128f = load(const, t["i128f"], name="I128f")
    I128h = load(const, t["i128h"], name="I128h")
    I32h = load(const, t["i32h"], name="I32h")
    ones = load(const, t["ones1"], name="ones")

    # index tiles
    qidx_sb = load(const, t["q_idx"].rearrange("(n p) -> n p", p=128)
                   .rearrange("n p -> p n"), name="qidx")   # [128, 13]
    aidx_sb = load(const, t["a_idx"].rearrange("(n p) -> n p", p=128)
                   .rearrange("n p -> p n"), name="aidx")   # [128, 64]

    # ---- embedding gather + transpose -> xT tiles --------------------
    def embed_T(table, idx_sb, ntiles, name):
        """gather rows (t-major) and transpose into xT [2 x [128, ntiles*128]] f16"""
        xT = [seqp.tile([128, ntiles * 128], F16, tag=f"{name}{k}", name=f"{name}{k}")
              for k in range(2)]
        for i in range(ntiles):
            rows = embp.tile([128, EMB], F16, tag="gather")
            nc.gpsimd.indirect_dma_start(
                out=rows[:], out_offset=None, in_=table[:],
                in_offset=IndirectOffsetOnAxis(ap=idx_sb[:, i:i + 1], axis=0))
            for k in range(2):
                p = ps_tr.tile([128, 128], F16, space="PSUM", tag="tr",
                               name="trp")
                nc.tensor.transpose(p[:], rows[:, 128 * k:128 * (k + 1)],
                                    I128h[:])
                nc.vector.tensor_copy(xT[k][:, 128 * i:128 * (i + 1)], p[:])
        return xT

    qT = embed_T(t["q_emb"], qidx_sb, 13, "qT")    # [256, 1664] f16
    aT = embed_T(t["a_emb"], aidx_sb, 64, "aT")    # [256, 8192] f16

    # ---- LSTM cell ---------------------------------------------------
    def step(hT, c_prev, seeds, wx_list, whh, has_h, want_hs):
        """One LSTM step, full-width col-tiled.

        seeds: list of (lhsT_ap[K,32], rhs_ap[K, 2048]) matmuls
        wx_list: list of (lhsT_ap, chunk_fn, k); whh: chunk accessor.
        """
        gp = ps_g.tile([128, 512], F32, space="PSUM", tag="gates")
        # rows = (lhsT_fn(sl), rhs_fn(sl)) emitted strip-innermost so the 4
        # col-strips run concurrently on the PE array
        rows = []
        for lhsT, rhs in seeds:
            rows.append((lambda sl, l=lhsT, r=rhs: (l, r[:, sl])))
        for lhsT, cf, k in wx_list:
            rows.append((lambda sl, l=lhsT, c2=cf, kk=k: (l, c2(kk, sl))))
        if has_h:
            for k in range(4):
                rows.append((lambda sl, kk=k: (hT[:, 32 * kk:32 * (kk + 1)],
                                               whh(kk, sl))))
        nrows = len(rows)
        for i, rowf in enumerate(rows):
            for c in range(4):
                sl = slice(512 * c, 512 * (c + 1))
                lhsT, rhs = rowf(sl)
                nc.tensor.matmul(gp[32 * c:32 * (c + 1), :], lhsT, rhs,
                                 start=(i == 0), stop=(i == nrows - 1),
                                 tile_position=(0, 32 * c))
        sig = ew.tile([128, 384], F32, tag="sig")   # i | f | o
        nc.scalar.activation(sig[:], gp[:, 0:384], AF.Sigmoid)
        gg = ew.tile([128, 128], F32, tag="g")
        nc.scalar.activation(gg[:], gp[:, 384:512], AF.Tanh)
        igg = ew.tile([128, 128], F32, tag="ig")
        nc.vector.tensor_mul(igg[:], sig[:, 0:128], gg[:])
        c_new = state.tile([128, 128], F32, tag="c")
        if c_prev is None:
            nc.vector.tensor_copy(c_new[:], igg[:])  # c0 = 0 -> c = i*g
        else:
            fc = ew.tile([128, 128], F32, tag="fc")
            nc.vector.tensor_mul(fc[:], sig[:, 128:256], c_prev[:])
            nc.vector.tensor_add(c_new[:], igg[:], fc[:])
        th = ew.tile([128, 128], F32, tag="th")
        nc.scalar.activation(th[:], c_new[:], AF.Tanh)
        h_new = ew.tile([128, 128], F16, tag="h")
        nc.vector.tensor_mul(h_new[:], sig[:, 256:384], th[:])
        trp = ps_tr.tile([128, 128], F16, space="PSUM", tag="tr", name="trh")
        nc.tensor.transpose(trp[:], h_new[:], I128h[:])
        hT_new = state.tile([128, 128], F16, tag="hT")
        nc.vector.tensor_copy(hT_new[:], trp[:])
        return hT_new, c_new

    # ---- encoder -----------------------------------------------------
    hT = None
    c = None
    for tt in range(LQ):
        sl32 = slice(32 * tt, 32 * (tt + 1))
        seeds = [(ones[:], b_e[:])]
        wx = [(qT[0][:, sl32], wih_e, 0),
              (qT[1][:, sl32], wih_e, 1)]
        hT, c = step(hT, c, seeds, wx, whh_e, has_h=(tt > 0), want_hs=False)

    # ---- q_out = h @ q_lin_w.T + b; then Qb = q_out @ w_ihQ.T + bias_dec
    qo_p_t = ps_m.tile([32, 512], F32, space="PSUM", tag="misc", name="qo_p")
    qo_p = qo_p_t[:, 0:QOUT]
    nc.tensor.matmul(qo_p[:], ones[:], qlb[:], start=True, stop=False)
    for k in range(4):
        nc.tensor.matmul(qo_p[:], hT[:, 32 * k:32 * (k + 1)],
                         qlw(k), start=False, stop=(k == 3))
    qo = seqp.tile([32, QOUT], F16, tag="qo_sb")
    nc.scalar.activation(qo[:], qo_p[:], AF.Identity)
    # transpose q_out [32,256] -> [256(2x128), 32] f16
    qoT = seqp.tile([128, 64], F16, tag="qoT")
    for k in range(2):
        p = ps_tr.tile([128, 128], F16, space="PSUM", tag="tr", name="trq")
        nc.tensor.transpose(p[:, 0:32], qo[:, 128 * k:128 * (k + 1)], I32h[:])
        nc.vector.tensor_copy(qoT[:, 32 * k:32 * (k + 1)], p[:, 0:32])
    # Qb [32, 2048] f16, quarter at a time (no col tiling, partition 0-31)
    qb = seqp.tile([32, G], F16, tag="qb")
    for qtr in range(4):
        sl = slice(512 * qtr, 512 * (qtr + 1))
        qp = ps_m.tile([32, 512], F32, space="PSUM", tag="misc")
        nc.tensor.matmul(qp[:], ones[:], b_d[:, sl], start=True, stop=False)
        for k in range(2):
            nc.tensor.matmul(qp[:], qoT[:, 32 * k:32 * (k + 1)],
                             wihQ(k, sl), start=False, stop=(k == 1))
        nc.scalar.activation(qb[:, sl], qp[:], AF.Identity)

    # ---- decoder + projection, interleaved per TBLK ------------------
    out = t["out"]
    for blk in range(STEPS // TBLK):
        hs = hsp.tile([128, 4 * TBLK * 32], F16, tag="hs", name="hs")
        for dt in range(TBLK):
            tt = blk * TBLK + dt
            sl32 = slice(32 * tt, 32 * (tt + 1))
            seeds = [(I32h[:], qb[:])]
            wx = [(aT[0][:, sl32], wihA, 0),
                  (aT[1][:, sl32], wihA, 1)]
            hT, c = step(hT, c, seeds, wx, whh_d, has_h=True, want_hs=True)
            # scatter hT into the hs block: col (k*32*TBLK + b*TBLK + dt)
            dst = hs.rearrange("p (k b t) -> p k b t", k=4, b=32)[:, :, :, dt]
            nc.gpsimd.tensor_copy(dst, hT[:].rearrange("p (k b) -> p k b", k=4))
        # projection of this block: tokens = 32*TBLK, b-major cols
        for m in range(VPAD // 128):
            for s in range(TBLK * 32 // 512):
                pp = ps_p.tile([128, 512], F32, space="PSUM", tag="proj")
                for k in range(4):
                    nc.tensor.matmul(
                        pp[:], linw(k, slice(128 * m, 128 * (m + 1))),
                        hs[:, TBLK * 32 * k + 512 * s:TBLK * 32 * k + 512 * (s + 1)],
                        start=(k == 0), stop=(k == 3))
                ot = outp.tile([128, 512], F32, tag="ot")
                if m % 2 == 0:
                    nc.scalar.activation(ot[:], pp[:], AF.Identity,
                                         bias=linb[:, m:m + 1])
                else:
                    nc.vector.tensor_scalar_add(ot[:], pp[:],
                                                linb[:, m:m + 1])
                nb = 512 // TBLK  # batches per sub-block
                dst = out[nb * s:nb * (s + 1), 128 * m:128 * (m + 1),
                          blk * TBLK:(blk + 1) * TBLK].rearrange(
                              "b w t -> w b t")
                nc.sync.dma_start(dst, ot[:].rearrange(
                    "w (b t) -> w b t", b=nb))


def kernel(**inputs):
    inp = {k: np.asarray(v) for k, v in inputs.items()}
    if "prog" not in _cache:
        _cache["prog"] = build_program()
    nc = _cache["prog"]

    perm = _gate_perm()
    f16 = np.float16

    def prep_lstm(w_ih, w_hh, b_ih, b_hh):
        wihT = np.ascontiguousarray(w_ih.T[:, perm]).astype(f16)
        whhT = np.ascontiguousarray(w_hh.T[:, perm]).astype(f16)
        bias = (b_ih + b_hh)[perm][None, :].astype(f16)
        return wihT, whhT, bias

    wihT_e, whhT_e, b_e = prep_lstm(inp["q_lstm_w_ih"], inp["q_lstm_w_hh"],
                                    inp["q_lstm_b_ih"], inp["q_lstm_b_hh"])
    wihT_d, whhT_d, b_d = prep_lstm(inp["a_lstm_w_ih"], inp["a_lstm_w_hh"],
                                    inp["a_lstm_b_ih"], inp["a_lstm_b_hh"])
    wihAT = np.ascontiguousarray(wihT_d[:EMB])
    wihQT = np.ascontiguousarray(wihT_d[EMB:])

    q_idx = np.zeros(13 * 128, np.int32)
    q_idx[:B * LQ] = inp["question"].T.reshape(-1).astype(np.int32)
    a_idx = inp["answer"][:, :STEPS].T.reshape(-1).astype(np.int32)

    lin_w = inp["lin_w"].astype(np.float32)   # [32000, 512]
    lin_b = inp["lin_b"].astype(np.float32)

    base = {
        "q_idx": q_idx, "a_idx": a_idx,
        "q_emb": inp["q_emb_w"].astype(f16),
        "a_emb": inp["a_emb_w"].astype(f16),
        "w_ihT_enc": wihT_e, "w_hhT_enc": whhT_e, "bias_enc": b_e,
        "w_ihAT": wihAT, "w_ihQT": wihQT, "w_hhT_dec": whhT_d,
        "bias_dec": b_d,
        "q_lin_wT": np.ascontiguousarray(inp["q_lin_w"].T).astype(f16),
        "q_lin_b": inp["q_lin_b"][None, :].astype(f16),
        "i128f": np.eye(128, dtype=np.float32),
        "i128h": np.eye(128, dtype=f16),
        "i32h": np.eye(32, dtype=f16),
        "ones1": np.ones((1, 32), f16),
    }
    in_maps = []
    for core in range(NCORES):
        m = dict(base)
        sl = lin_w[VSH * core: VSH * (core + 1)]          # [4000, 512]
        slp = np.zeros((VPAD, HID), np.float32)
        slp[:VSH] = sl
        m["lin_wT"] = np.ascontiguousarray(slp.T).astype(f16)
        bp = np.zeros(VPAD, np.float32)
        bp[:VSH] = lin_b[VSH * core: VSH * (core + 1)]
        m["lin_b"] = np.ascontiguousarray(bp.reshape(32, 128).T)
        in_maps.append(m)

    _cache["in_maps"] = in_maps
    res = run_bass_kernel_spmd(nc, in_maps, core_ids=list(range(NCORES)))
    _cache["last_res"] = res
    out = np.concatenate(
        [res.results[i]["out"][:, :VSH, :] for i in range(NCORES)], axis=1)
    return out.astype(np.float32)


if __name__ == "__main__":
    import reference
    ins = reference.setup_inputs()
    ref = np.asarray(reference.reference(**ins))
    got = kernel(**{k: np.asarray(v) for k, v in ins.items()})
    err = np.abs(got - ref).max() / (np.abs(ref).max() + 1e-12)
    print("max abs err:", np.abs(got - ref).max(), "rel:", err)


def run_traced():
    nc = _cache["prog"]
    return run_bass_kernel_spmd(nc, _cache["in_maps"],
                                core_ids=list(range(NCORES)), trace=True)


# revision 20
# speedup vs baseline: 1.1601x; 1.0800x over previous
"""Trainium2 Bass kernel for an LSTM encoder-decoder chatbot model.

Model: question -> embed -> LSTM(512) -> linear(256) = q_out
       answer[:, :256] -> embed -> concat(q_out) -> LSTM(512) -> linear(32000)
Output: logits [B=32, W=32000, STEPS=256] f32.

Sharding: all 8 cores run the full (replicated) encoder + decoder
recurrence; the dominant 512x32000 output projection is sharded
column-wise (vocab) across cores; each core emits [32, 4000, 256].

Matmul strategy: hidden state kept transposed (hT fp16 [128, 4x32])
as the PE stationary operand; weights stream as the moving operand in
fp16. Four col-tiled matmuls (tile_position=(0,32c)) run concurrently,
one per 512-unit gate block, so the gates land on all 128 PSUM
partitions [(block,b), 512=i|f|g|o] and the elementwise LSTM cell runs
full-width. Gate columns are host-permuted accordingly.
"""
import sys
import numpy as np

sys.path.insert(0, '/opt/trn_rl_repo')

import concourse.bass as bass  # noqa: E402
import concourse.bacc as bacc  # noqa: E402
import concourse.mybir as mybir  # noqa: E402
import concourse.tile as tile  # noqa: E402
from concourse.bass import IndirectOffsetOnAxis  # noqa: E402
from concourse.bass_utils import run_bass_kernel_spmd  # noqa: E402

F32 = mybir.dt.float32
F16 = mybir.dt.float16
I32DT = mybir.dt.int32
AF = mybir.ActivationFunctionType

W_VOCAB = 32000
EMB = 256
STEPS = 256
HID = 512
QOUT = 256
B = 32
LQ = 50
NCORES = 8
VSH = W_VOCAB // NCORES      # 4000 vocab rows per core
VPAD = 4096                   # padded to 32 tiles of 128
G = 4 * HID                   # 2048 gate columns
TBLK = 32                     # decoder steps per hs block (8 blocks)

_cache = {}


def _gate_perm():
    """Block layout [i|f|o|g]x128 per 128-unit block: new col
    j = 512*blk + 128*slot + u  <-  old row 512*gate + 128*blk + u,
    with slot order (i, f, o, g) so the three sigmoids are contiguous."""
    j = np.arange(G)
    blk, r = j // 512, j % 512
    slot, u = r // 128, r % 128
    old_gate = np.array([0, 1, 3, 2])[slot]
    return 512 * old_gate + 128 * blk + u


def build_program():
    nc = bacc.Bacc("TRN2", target_bir_lowering=False, debug=False,
                   num_devices=NCORES)

    def inp(name, shape, dt):
        return nc.dram_tensor(name, shape, dt, kind="ExternalInput").ap()

    q_idx = inp("q_idx", [13 * 128], I32DT)            # padded 1664
    a_idx = inp("a_idx", [STEPS * B], I32DT)           # 8192, t-major
    q_emb = inp("q_emb", [W_VOCAB, EMB], F16)
    a_emb = inp("a_emb", [W_VOCAB, EMB], F16)
    w_ihT_enc = inp("w_ihT_enc", [EMB, G], F16)        # permuted cols
    w_hhT_enc = inp("w_hhT_enc", [HID, G], F16)
    bias_enc = inp("bias_enc", [1, G], F16)
    w_ihAT = inp("w_ihAT", [EMB, G], F16)
    w_ihQT = inp("w_ihQT", [QOUT, G], F16)
    w_hhT_dec = inp("w_hhT_dec", [HID, G], F16)
    bias_dec = inp("bias_dec", [1, G], F16)
    q_lin_wT = inp("q_lin_wT", [HID, QOUT], F16)
    q_lin_b = inp("q_lin_b", [1, QOUT], F16)
    lin_wT = inp("lin_wT", [HID, VPAD], F16)           # per-core slice
    lin_b = inp("lin_b", [128, 32], F32)               # [u, mtile]
    i128f = inp("i128f", [128, 128], F32)
    i128h = inp("i128h", [128, 128], F16)
    i32h = inp("i32h", [32, 32], F16)
    ones1 = inp("ones1", [1, 32], F16)
    out = nc.dram_tensor("out", [B, VPAD, STEPS], F32,
                         kind="ExternalOutput").ap()

    with tile.TileContext(nc) as tc:
        _build(nc, tc, locals())
    nc.compile()
    return nc


def _build(nc, tc, t):
    from contextlib import ExitStack
    ctx = ExitStack()
    with ctx:
        _build_inner(nc, tc, t, ctx)


def _build_inner(nc, tc, t, ctx):
    # ---- pools -------------------------------------------------------
    wpool = ctx.enter_context(tc.tile_pool(name="weights", bufs=1))
    const = ctx.enter_context(tc.tile_pool(name="const", bufs=1))
    embp = ctx.enter_context(tc.tile_pool(name="embp", bufs=6))
    seqp = ctx.enter_context(tc.tile_pool(name="seqp", bufs=1))
    state = ctx.enter_context(tc.tile_pool(name="state", bufs=3))
    ew = ctx.enter_context(tc.tile_pool(name="ew", bufs=3))
    hsp = ctx.enter_context(tc.tile_pool(name="hsp", bufs=3))
    outp = ctx.enter_context(tc.tile_pool(name="outp", bufs=6))
    ps_g = ctx.enter_context(tc.tile_pool(name="ps_g", bufs=3, space="PSUM"))
    ps_tr = ctx.enter_context(tc.tile_pool(name="ps_tr", bufs=2, space="PSUM"))
    ps_p = ctx.enter_context(tc.tile_pool(name="ps_p", bufs=3, space="PSUM"))

    def load(pool, ap, dt=None, name=None):
        s = pool.tile(list(ap.shape), dt or ap.dtype, tag=name, name=name or 'ld')
        nc.sync.dma_start(s[:], ap[:])
        return s

    def loadc(pool, ap, name):
        p, cdim = ap.shape
        n = p // 128
        s = pool.tile([128, n * cdim], ap.dtype, tag=name, name=name)
        for k in range(n):
            nc.sync.dma_start(s[:, cdim * k:cdim * (k + 1)],
                              ap[128 * k:128 * (k + 1), :])
        def chunk(k, sl=slice(None)):
            base = cdim * k
            if sl == slice(None):
                return s[:, base:base + cdim]
            return s[:, base + sl.start:base + sl.stop]
        return chunk

    # ---- resident weights/constants ---------------------------------
    wih_e = loadc(wpool, t["w_ihT_enc"], "wih_e")     # 2 chunks [128,2048]
    whh_e = loadc(wpool, t["w_hhT_enc"], "whh_e")     # 4 chunks
    b_e = load(const, t["bias_enc"], name="b_e")
    wihA = loadc(wpool, t["w_ihAT"], "wihA")
    wihQ = loadc(wpool, t["w_ihQT"], "wihQ")
    whh_d = loadc(wpool, t["w_hhT_dec"], "whh_d")
    b_d = load(const, t["bias_dec"], name="b_d")
    qlw = loadc(wpool, t["q_lin_wT"], "qlw")          # 4 chunks [128,256]
    qlb = load(const, t["q_lin_b"], name="qlb")
    linw = loadc(wpool, t["lin_wT"], "linw")          # 4 chunks [128,4096]
    linb = load(const, t["lin_b"], name="linb")           # [128, 32] f32
    I128f = load(const, t["i128f"], name="I128f")
    I128h = load(const, t["i128h"], name="I128h")
    I32h = load(const, t["i32h"], name="I32h")
    ones = load(const, t["ones1"], name="ones")

    # index tiles
    qidx_sb = load(const, t["q_idx"].rearrange("(n p) -> n p", p=128)
                   .rearrange("n p -> p n"), name="qidx")   # [128, 13]
    aidx_sb = load(const, t["a_idx"].rearrange("(n p) -> n p", p=128)
                   .rearrange("n p -> p n"), name="aidx")   # [128, 64]

    # ---- embedding gather + transpose -> xT tiles --------------------
    def embed_T(table, idx_sb, ntiles, name):
        """gather rows (t-major) and transpose into xT [2 x [128, ntiles*128]] f16"""
        xT = [seqp.tile([128, ntiles * 128], F16, tag=f"{name}{k}", name=f"{name}{k}")
              for k in range(2)]
        for i in range(ntiles):
            rows = embp.tile([128, EMB], F16, tag="gather")
            nc.gpsimd.indirect_dma_start(
                out=rows[:], out_offset=None, in_=table[:],
                in_offset=IndirectOffsetOnAxis(ap=idx_sb[:, i:i + 1], axis=0))
            for k in range(2):
                p = ps_tr.tile([128, 128], F16, space="PSUM", tag="tr",
                               name="trp")
                nc.tensor.transpose(p[:], rows[:, 128 * k:128 * (k + 1)],
                                    I128h[:])
                nc.vector.tensor_copy(xT[k][:, 128 * i:128 * (i + 1)], p[:])
        return xT

    qT = embed_T(t["q_emb"], qidx_sb, 13, "qT")    # [256, 1664] f16
    # aT emission is deferred: tiles created now, per-tile gather+transpose
    # emitted interleaved into the encoder steps to fill PE chain gaps.
    aT = [seqp.tile([128, 64 * 128], F16, tag=f"aT{k}", name=f"aT{k}")
          for k in range(2)]

    def emit_aT(i):
        rows = embp.tile([128, EMB], F16, tag="gather", name="arows")
        nc.gpsimd.indirect_dma_start(
            out=rows[:], out_offset=None, in_=t["a_emb"][:],
            in_offset=IndirectOffsetOnAxis(ap=aidx_sb[:, i:i + 1], axis=0))
        for k in range(2):
            p = ps_tr.tile([128, 128], F16, space="PSUM", tag="tr",
                           name="trpa")
            nc.tensor.transpose(p[:], rows[:, 128 * k:128 * (k + 1)],
                                I128h[:])
            nc.vector.tensor_copy(aT[k][:, 128 * i:128 * (i + 1)], p[:])

    # ---- LSTM cell ---------------------------------------------------
    def step(hT, c_prev, seeds, wx_list, whh, has_h, want_hs):
        """One LSTM step, full-width col-tiled.

        seeds: list of (lhsT_ap[K,32], rhs_ap[K, 2048]) matmuls
        wx_list: list of (lhsT_ap, chunk_fn, k); whh: chunk accessor.
        """
        gp = ps_g.tile([128, 512], F32, space="PSUM", tag="gates")
        # rows = (lhsT_fn(sl), rhs_fn(sl)) emitted strip-innermost so the 4
        # col-strips run concurrently on the PE array
        rows = []
        for lhsT, rhs in seeds:
            rows.append((lambda sl, l=lhsT, r=rhs: (l, r[:, sl])))
        for lhsT, cf, k in wx_list:
            rows.append((lambda sl, l=lhsT, c2=cf, kk=k: (l, c2(kk, sl))))
        if has_h:
            for k in range(4):
                rows.append((lambda sl, kk=k: (hT[:, 32 * kk:32 * (kk + 1)],
                                               whh(kk, sl))))
        nrows = len(rows)
        for i, rowf in enumerate(rows):
            for c in range(4):
                sl = slice(512 * c, 512 * (c + 1))
                lhsT, rhs = rowf(sl)
                nc.tensor.matmul(gp[32 * c:32 * (c + 1), :], lhsT, rhs,
                                 start=(i == 0), stop=(i == nrows - 1),
                                 tile_position=(0, 32 * c))
        sig = ew.tile([128, 384], F32, tag="sig")   # i | f | o
        nc.scalar.activation(sig[:], gp[:, 0:384], AF.Sigmoid)
        gg = ew.tile([128, 128], F32, tag="g")
        nc.scalar.activation(gg[:], gp[:, 384:512], AF.Tanh)
        igg = ew.tile([128, 128], F32, tag="ig")
        nc.vector.tensor_mul(igg[:], sig[:, 0:128], gg[:])
        c_new = state.tile([128, 128], F32, tag="c")
        if c_prev is None:
            nc.vector.tensor_copy(c_new[:], igg[:])  # c0 = 0 -> c = i*g
        else:
            fc = ew.tile([128, 128], F32, tag="fc")
            nc.vector.tensor_mul(fc[:], sig[:, 128:256], c_prev[:])
            nc.vector.tensor_add(c_new[:], igg[:], fc[:])
        th = ew.tile([128, 128], F32, tag="th")
        nc.scalar.activation(th[:], c_new[:], AF.Tanh)
        h_new = ew.tile([128, 128], F16, tag="h")
        nc.vector.tensor_mul(h_new[:], sig[:, 256:384], th[:])
        trp = ps_tr.tile([128, 128], F16, space="PSUM", tag="tr", name="trh")
        nc.tensor.transpose(trp[:], h_new[:], I128h[:])
        hT_new = state.tile([128, 128], F16, tag="hT")
        nc.vector.tensor_copy(hT_new[:], trp[:])
        return hT_new, c_new

    # ---- encoder -----------------------------------------------------
    hT = None
    c = None
    a_emitted = 0
    for tt in range(LQ):
        sl32 = slice(32 * tt, 32 * (tt + 1))
        seeds = [(ones[:], b_e[:])]
        wx = [(qT[0][:, sl32], wih_e, 0),
              (qT[1][:, sl32], wih_e, 1)]
        hT, c = step(hT, c, seeds, wx, whh_e, has_h=(tt > 0), want_hs=False)
        want = (tt + 1) * 64 // LQ
        while a_emitted < want:
            emit_aT(a_emitted)
            a_emitted += 1
    while a_emitted < 64:
        emit_aT(a_emitted)
        a_emitted += 1

    # ---- q_out = h @ q_lin_w.T + b; then Qb = q_out @ w_ihQ.T + bias_dec
    qo_p_t = ps_p.tile([128, 512], F32, space="PSUM", tag="proj", name="qo_p")
    qo_p = qo_p_t[0:32, 0:QOUT]
    nc.tensor.matmul(qo_p[:], ones[:], qlb[:], start=True, stop=False)
    for k in range(4):
        nc.tensor.matmul(qo_p[:], hT[:, 32 * k:32 * (k + 1)],
                         qlw(k), start=False, stop=(k == 3))
    qo = seqp.tile([32, QOUT], F16, tag="qo_sb")
    nc.scalar.activation(qo[:], qo_p[:], AF.Identity)
    # transpose q_out [32,256] -> [256(2x128), 32] f16
    qoT = seqp.tile([128, 64], F16, tag="qoT")
    for k in range(2):
        p = ps_tr.tile([128, 128], F16, space="PSUM", tag="tr", name="trq")
        nc.tensor.transpose(p[:, 0:32], qo[:, 128 * k:128 * (k + 1)], I32h[:])
        nc.vector.tensor_copy(qoT[:, 32 * k:32 * (k + 1)], p[:, 0:32])
    # Qb [32, 2048] f16, quarter at a time (no col tiling, partition 0-31)
    qb = seqp.tile([32, G], F16, tag="qb")
    for qtr in range(4):
        sl = slice(512 * qtr, 512 * (qtr + 1))
        qp = ps_p.tile([128, 512], F32, space="PSUM", tag="proj", name="qp")[0:32, :]
        nc.tensor.matmul(qp[:], ones[:], b_d[:, sl], start=True, stop=False)
        for k in range(2):
            nc.tensor.matmul(qp[:], qoT[:, 32 * k:32 * (k + 1)],
                             wihQ(k, sl), start=False, stop=(k == 1))
        nc.scalar.activation(qb[:, sl], qp[:], AF.Identity)

    # ---- decoder + projection, software-pipelined --------------------
    # Block b's 32 vocab-tile projections are emitted one per step during
    # block b+1's recurrence, filling PE gaps in the chain-bound LSTM.
    out = t["out"]

    def proj_m(hs, blk, m):
        for s in range(TBLK * 32 // 512):
            pp = ps_p.tile([128, 512], F32, space="PSUM", tag="proj")
            for k in range(4):
                nc.tensor.matmul(
                    pp[:], linw(k, slice(128 * m, 128 * (m + 1))),
                    hs[:, TBLK * 32 * k + 512 * s:TBLK * 32 * k + 512 * (s + 1)],
                    start=(k == 0), stop=(k == 3))
            ot = outp.tile([128, 512], F32, tag="ot")
            if m % 2 == 0:
                nc.scalar.activation(ot[:], pp[:], AF.Identity,
                                     bias=linb[:, m:m + 1])
            else:
                nc.vector.tensor_scalar_add(ot[:], pp[:], linb[:, m:m + 1])
            nb = 512 // TBLK  # batches per sub-block
            dst = out[nb * s:nb * (s + 1), 128 * m:128 * (m + 1),
                      blk * TBLK:(blk + 1) * TBLK].rearrange("b w t -> w b t")
            nc.sync.dma_start(dst, ot[:].rearrange("w (b t) -> w b t", b=nb))

    hs_prev = None
    for blk in range(STEPS // TBLK):
        hs = hsp.tile([128, 4 * TBLK * 32], F16, tag="hs", name="hs")
        for dt in range(TBLK):
            tt = blk * TBLK + dt
            sl32 = slice(32 * tt, 32 * (tt + 1))
            seeds = [(I32h[:], qb[:])]
            wx = [(aT[0][:, sl32], wihA, 0),
                  (aT[1][:, sl32], wihA, 1)]
            hT, c = step(hT, c, seeds, wx, whh_d, has_h=True, want_hs=True)
            # scatter hT into the hs block: col (k*32*TBLK + b*TBLK + dt)
            dst = hs.rearrange("p (k b t) -> p k b t", k=4, b=32)[:, :, :, dt]
            nc.gpsimd.tensor_copy(dst, hT[:].rearrange("p (k b) -> p k b", k=4))
            if hs_prev is not None:
                proj_m(hs_prev, blk - 1, dt)
        hs_prev = hs
    for m in range(VPAD // 128):
        proj_m(hs_prev, STEPS // TBLK - 1, m)


def kernel(**inputs):
    inp = {k: np.asarray(v) for k, v in inputs.items()}
    if "prog" not in _cache:
        _cache["prog"] = build_program()
    nc = _cache["prog"]

    perm = _gate_perm()
    f16 = np.float16

    def prep_lstm(w_ih, w_hh, b_ih, b_hh):
        wihT = np.ascontiguousarray(w_ih.T[:, perm]).astype(f16)
        whhT = np.ascontiguousarray(w_hh.T[:, perm]).astype(f16)
        bias = (b_ih + b_hh)[perm][None, :].astype(f16)
        return wihT, whhT, bias

    wihT_e, whhT_e, b_e = prep_lstm(inp["q_lstm_w_ih"], inp["q_lstm_w_hh"],
                                    inp["q_lstm_b_ih"], inp["q_lstm_b_hh"])
    wihT_d, whhT_d, b_d = prep_lstm(inp["a_lstm_w_ih"], inp["a_lstm_w_hh"],
                                    inp["a_lstm_b_ih"], inp["a_lstm_b_hh"])
    wihAT = np.ascontiguousarray(wihT_d[:EMB])
    wihQT = np.ascontiguousarray(wihT_d[EMB:])

    q_idx = np.zeros(13 * 128, np.int32)
    q_idx[:B * LQ] = inp["question"].T.reshape(-1).astype(np.int32)
    a_idx = inp["answer"][:, :STEPS].T.reshape(-1).astype(np.int32)

    lin_w = inp["lin_w"].astype(np.float32)   # [32000, 512]
    lin_b = inp["lin_b"].astype(np.float32)

    base = {
        "q_idx": q_idx, "a_idx": a_idx,
        "q_emb": inp["q_emb_w"].astype(f16),
        "a_emb": inp["a_emb_w"].astype(f16),
        "w_ihT_enc": wihT_e, "w_hhT_enc": whhT_e, "bias_enc": b_e,
        "w_ihAT": wihAT, "w_ihQT": wihQT, "w_hhT_dec": whhT_d,
        "bias_dec": b_d,
        "q_lin_wT": np.ascontiguousarray(inp["q_lin_w"].T).astype(f16),
        "q_lin_b": inp["q_lin_b"][None, :].astype(f16),
        "i128f": np.eye(128, dtype=np.float32),
        "i128h": np.eye(128, dtype=f16),
        "i32h": np.eye(32, dtype=f16),
        "ones1": np.ones((1, 32), f16),
    }
    in_maps = []
    for core in range(NCORES):
        m = dict(base)
        sl = lin_w[VSH * core: VSH * (core + 1)]          # [4000, 512]
        slp = np.zeros((VPAD, HID), np.float32)
        slp[:VSH] = sl
        m["lin_wT"] = np.ascontiguousarray(slp.T).astype(f16)
        bp = np.zeros(VPAD, np.float32)
        bp[:VSH] = lin_b[VSH * core: VSH * (core + 1)]
        m["lin_b"] = np.ascontiguousarray(bp.reshape(32, 128).T)
        in_maps.append(m)

    _cache["in_maps"] = in_maps
    res = run_bass_kernel_spmd(nc, in_maps, core_ids=list(range(NCORES)))
    _cache["last_res"] = res
    out = np.concatenate(
        [res.results[i]["out"][:, :VSH, :] for i in range(NCORES)], axis=1)
    return out.astype(np.float32)


if __name__ == "__main__":
    import reference
    ins = reference.setup_inputs()
    ref = np.asarray(reference.reference(**ins))
    got = kernel(**{k: np.asarray(v) for k, v in ins.items()})
    err = np.abs(got - ref).max() / (np.abs(ref).max() + 1e-12)
    print("max abs err:", np.abs(got - ref).max(), "rel:", err)


def run_traced():
    nc = _cache["prog"]
    return run_bass_kernel_spmd(nc, _cache["in_maps"],
                                core_ids=list(range(NCORES)), trace=True)
